# revision 1
# baseline (speedup 1.0000x reference)
"""Trainium2 Bass kernel for nn_AttentionModule_39616778338491 (chord sparse attention).

Structure: V = gMLP(V); 12x { W = fMLP_m(input); V = chord_spmm(W, V) + V }.

Sharding (8 cores): core c -> batch b=c//2, row-half h=c%2 for the MLPs
(data-parallel); per-pair AllGather shares W and g(V); the chord chain runs
replicated within each pair (full 4096 rows) so no per-layer halo exchange
is needed.

Program layout: TileContext 1 (g + 12 f-MLPs, bf16 matmuls, staging to DRAM)
-> raw block with 13 pair-AllGathers (Tile's collective path deadlocks on
this target; raw gpsimd collectives with Local outputs work) -> all-engine
barrier -> TileContext 2 (chord chain).

Chord spmm as dense PE matmuls: for each 128-row output block the 13
power-of-2 offsets touch 6 source blocks {+0,+1,+2,+4,+8,+16}; per-row
weights are embedded as diagonals of six 128x128 bf16 lhsT tiles, rebuilt
per layer by skewed flat DMAs into a DRAM staging image (diagonal writes
couple partition and byte offsets, which only DRAM-side APs allow) and
reloaded into double-buffered SBUF tiles. The +V residual is an exact-f32
vector-engine add at PSUM evacuation.
"""

import os
import numpy as np

B, N, E, H = 4, 4096, 256, 1024
NW = 12
NL = 13
OFFS = [0, 1, 2, 4, 8, 16, 32, 64, 128, 256, 512, 1024, 2048]
ROWS = N // 2          # rows per core for MLP work
NBLK = N // 128        # 32 blocks of 128 rows
CH = 512               # row-chunk for MLP matmuls
NCH = ROWS // CH
HT = H // 128          # 8 h-tiles
PITCH = NBLK * 128     # free width of an S tile (elems)
GROUPS = [[0, 1], [2, 3], [4, 5], [6, 7]]


def _install_patches():
    """Walrus in this image rejects >1 sem wait on the Tile tail Drain;
    spread the waits across preceding sync-engine nops. Also raise the
    stale SBUF cap (207.87 KB/partition is the real limit here)."""
    import concourse.mybir as mybir
    from concourse.tile import TileContext
    from concourse.vector_clock import ScopedClock
    from concourse import tile_utils

    def _dab(self, tick_clock, wait_clock):
        nops = [self.nc.sync.nop(nofuse=True) for _ in range(27)]
        drain_inst = self.nc.sync.drain()
        wait_clock.add_sem_waits(
            drain_inst.ins, ScopedClock({None: tick_clock.global_clock})
        )
        si = drain_inst.ins.sync_info
        waits = list(si.on_wait) if si else []
        if len(waits) > 1:
            si.on_wait.clear()
            si.on_wait.append(waits[0])
            for w, nop in zip(waits[1:], nops):
                nsi = nop.ins.sync_info
                if nsi is None:
                    nop.ins.sync_info = mybir.SyncInfo(on_update=[], on_wait=[w])
                else:
                    nsi.on_wait.append(w)
        self.nc.all_engine_barrier()
        popped = self.nc._tile_sem_poison_stack.pop()
        assert popped is self._sem_poison
        self.nc.clear_and_free_semaphores(list(self.sems.allocated().values()))
        self.nc.all_engine_barrier()

    TileContext._drain_and_barrier = _dab
    tile_utils.max_sbuf_usage = 206 * 1024


def _split_multi_waits(nc, mybir, limit=1):
    """This walrus build accepts at most one sem wait per instruction;
    hoist extra waits onto same-engine NoOps inserted just before."""
    uid = 0
    for f in nc.m.functions:
        for bb in f.blocks:
            new = []
            for inst in bb.instructions:
                si = inst.sync_info
                waits = list(si.on_wait) if si and si.on_wait else []
                if len(waits) > limit:
                    for w in waits[:-limit]:
                        nop = mybir.InstNoOp(name=f"waitsplit-{uid}", ins=[], outs=[])
                        uid += 1
                        nop.engine = inst.engine
                        nop.sync_info = mybir.SyncInfo(on_update=[], on_wait=[w])
                        new.append(nop)
                    si.on_wait.clear()
                    si.on_wait.append(waits[-1])
                new.append(inst)
            bb.instructions = new


def _build_program(nw):
    import bass_rust
    import concourse.bass as bass
    import concourse.mybir as mybir
    from concourse.tile import TileContext

    f32 = mybir.dt.float32
    bf16 = mybir.dt.bfloat16
    AF = mybir.ActivationFunctionType
    V64 = bass_rust.VecI64Pair

    nc = bass.Bass()
    vt = nc.declare_dram_parameter("vt", [E, ROWS], bf16, isOutput=False)
    inpt = nc.declare_dram_parameter("inpt", [E, ROWS], bf16, isOutput=False)
    gw1 = nc.declare_dram_parameter("gw1", [E, H], bf16, isOutput=False)
    gw2 = nc.declare_dram_parameter("gw2", [H, E], bf16, isOutput=False)
    gb1t = nc.declare_dram_parameter("gb1t", [128, HT], f32, isOutput=False)
    gb2r = nc.declare_dram_parameter("gb2r", [1, E], bf16, isOutput=False)
    fw1 = nc.declare_dram_parameter("fw1", [nw, E, H], bf16, isOutput=False)
    fw2t = nc.declare_dram_parameter("fw2t", [nw, 128, HT * NL], bf16, isOutput=False)
    fb1t = nc.declare_dram_parameter("fb1t", [128, nw * HT], f32, isOutput=False)
    fb2c = nc.declare_dram_parameter("fb2c", [NL, nw], f32, isOutput=False)
    onesr = nc.declare_dram_parameter("onesr", [1, E], bf16, isOutput=False)
    out = nc.declare_dram_parameter("out", [N, E], f32, isOutput=True)

    # raw DRAM staging (crosses the TileContext boundary; the phase barrier
    # orders accesses)
    vstage_in = nc.dram_tensor("vstage_in", [ROWS, E], f32)
    vstage_out = nc.dram_tensor("vstage_out", [2, ROWS, E], f32)
    wsis = [nc.dram_tensor(f"wsi{m}", [NL, ROWS], bf16) for m in range(nw)]
    wsos = [nc.dram_tensor(f"wso{m}", [2, NL, ROWS], bf16) for m in range(nw)]
    stage = [nc.dram_tensor(f"sst{p}", [6 * 128 * PITCH], bf16) for p in range(2)]

    # ---------------- phase 1: MLPs ----------------
    with TileContext(nc) as tc:
        with (
            tc.tile_pool(name="pc", bufs=1) as pc,
            tc.tile_pool(name="pin", bufs=1) as pin,
            tc.tile_pool(name="pfh", bufs=1) as pfh,
            tc.tile_pool(name="pfw1", bufs=2) as pfw1,
            tc.tile_pool(name="pfw2", bufs=2) as pfw2,
            tc.tile_pool(name="pvtc", bufs=2) as pvtc,
            tc.tile_pool(name="ptmp", bufs=3) as ptmp,
            tc.tile_pool(name="psA", bufs=3, space="PSUM") as psA,
            tc.tile_pool(name="psW", bufs=2, space="PSUM") as psW,
            tc.tile_pool(name="psO", bufs=3, space="PSUM") as psO,
        ):
            gw1_t = [pc.tile([128, H], bf16, tag=f"gw1_{k}", name=f"gw1_{k}") for k in range(2)]
            gw2_t = pc.tile([128, HT * E], bf16, tag="gw2", name="gw2")
            gb1_t = pc.tile([128, HT], f32, tag="gb1", name="gb1")
            gb2_t = pc.tile([1, E], bf16, tag="gb2", name="gb2")
            ones_t = pc.tile([1, E], bf16, tag="ones", name="ones")
            fb1_t = pc.tile([128, nw * HT], f32, tag="fb1", name="fb1")
            fb2_t = pc.tile([NL, nw], f32, tag="fb2", name="fb2")
            inp_t = [pin.tile([128, ROWS], bf16, tag=f"inp{k}", name=f"inp{k}") for k in range(2)]
            zt = pc.tile([128, PITCH], bf16, tag="zt", name="zt")

            for k in range(2):
                nc.sync.dma_start(out=gw1_t[k][:], in_=gw1[k * 128:(k + 1) * 128, :])
                nc.sync.dma_start(out=inp_t[k][:], in_=inpt[k * 128:(k + 1) * 128, :])
            for t in range(HT):
                nc.sync.dma_start(
                    out=gw2_t[:, t * E:(t + 1) * E], in_=gw2[t * 128:(t + 1) * 128, :]
                )
            nc.sync.dma_start(out=gb1_t[:], in_=gb1t[:])
            nc.sync.dma_start(out=gb2_t[:], in_=gb2r[:])
            nc.sync.dma_start(out=ones_t[:], in_=onesr[:])
            nc.sync.dma_start(out=fb1_t[:], in_=fb1t[:])
            nc.sync.dma_start(out=fb2_t[:], in_=fb2c[:])

            # zero the S staging images once (diagonal rewrites never touch
            # the off-diagonal zeros again)
            nc.vector.memset(zt[:], 0.0)
            for par in range(2):
                for k in range(6):
                    nc.sync.dma_start(
                        out=stage[par][k * 128 * PITCH:(k + 1) * 128 * PITCH].rearrange(
                            "(p f) -> p f", f=PITCH
                        ),
                        in_=zt[:],
                    )

            def mlp_front(w1k, bias_col, rhs_tiles):
                fh = [pfh.tile([128, CH], bf16, tag=f"fh{t}", name=f"fh{t}") for t in range(HT)]
                for ht in range(HT):
                    pa = psA.tile([128, CH], f32, tag="pa", name="pa")
                    for kt in range(2):
                        nc.tensor.matmul(
                            pa[:],
                            lhsT=w1k[kt][:, ht * 128:(ht + 1) * 128],
                            rhs=rhs_tiles[kt],
                            start=(kt == 0),
                            stop=(kt == 1),
                        )
                    nc.scalar.activation(fh[ht][:], pa[:], AF.Gelu, bias=bias_col(ht))
                return fh

            # g MLP -> vstage_in (own rows, row-major, f32)
            for ch in range(NCH):
                vt_c = [pvtc.tile([128, CH], bf16, tag=f"vtc{k}", name=f"vtc{k}") for k in range(2)]
                for k in range(2):
                    nc.sync.dma_start(
                        out=vt_c[k][:], in_=vt[k * 128:(k + 1) * 128, ch * CH:(ch + 1) * CH]
                    )
                fh = mlp_front(gw1_t, lambda ht: gb1_t[:, ht:ht + 1], vt_c)
                for t in range(4):
                    po = psO.tile([128, E], f32, tag="po", name="po")
                    nc.tensor.matmul(
                        po[:], lhsT=ones_t[0:1, 0:128], rhs=gb2_t[0:1, :],
                        start=True, stop=False,
                    )
                    for ht in range(HT):
                        nc.tensor.matmul(
                            po[:],
                            lhsT=fh[ht][:, t * 128:(t + 1) * 128],
                            rhs=gw2_t[:, ht * E:(ht + 1) * E],
                            start=False,
                            stop=(ht == HT - 1),
                        )
                    tmp = ptmp.tile([128, E], f32, tag="tv", name="tv")
                    nc.scalar.copy(tmp[:], po[:])
                    blk = ch * 4 + t
                    nc.sync.dma_start(
                        out=vstage_in[blk * 128:(blk + 1) * 128, :], in_=tmp[:]
                    )

            # f MLPs -> wsis[m] (own rows, [NL, ROWS], bf16)
            for m in range(nw):
                w1 = [pfw1.tile([128, H], bf16, tag=f"fw1_{k}", name=f"fw1_{k}") for k in range(2)]
                for k in range(2):
                    nc.sync.dma_start(out=w1[k][:], in_=fw1[m, k * 128:(k + 1) * 128, :])
                w2 = pfw2.tile([128, HT * NL], bf16, tag="fw2", name="fw2")
                nc.sync.dma_start(out=w2[:], in_=fw2t[m])
                for ch in range(NCH):
                    rhs = [inp_t[k][:, ch * CH:(ch + 1) * CH] for k in range(2)]
                    fh = mlp_front(
                        w1, lambda ht: fb1_t[:, m * HT + ht:m * HT + ht + 1], rhs
                    )
                    pw = psW.tile([NL, CH], f32, tag="pw", name="pw")
                    for ht in range(HT):
                        nc.tensor.matmul(
                            pw[:],
                            lhsT=w2[:, ht * NL:(ht + 1) * NL],
                            rhs=fh[ht][:],
                            start=(ht == 0),
                            stop=(ht == HT - 1),
                        )
                    wc = ptmp.tile([NL, CH], bf16, tag="tw", name="tw")
                    nc.vector.tensor_scalar_add(wc[:], pw[:], fb2_t[:, m:m + 1])
                    nc.sync.dma_start(
                        out=wsis[m][:, ch * CH:(ch + 1) * CH], in_=wc[:]
                    )

    # ---------------- raw pair-AllGathers ----------------
    with (
        nc.semaphore("ag_sem") as ag_sem,
        nc.Block() as blk,
    ):
        @blk.gpsimd
        def _(g):
            g.collective_compute(
                "AllGather", mybir.AluOpType.bypass, replica_groups=GROUPS,
                ins=[vstage_in[:]], outs=[vstage_out[:]],
            ).then_inc(ag_sem)
            for m in range(nw):
                g.collective_compute(
                    "AllGather", mybir.AluOpType.bypass, replica_groups=GROUPS,
                    ins=[wsis[m][:]], outs=[wsos[m][:]],
                ).then_inc(ag_sem)
            g.wait_ge(ag_sem, nw + 1)

    nc.all_engine_barrier()

    # ---------------- phase 2: chord chain ----------------
    with TileContext(nc) as tc:
        with (
            tc.tile_pool(name="pv", bufs=1) as pv,
            tc.tile_pool(name="ps", bufs=1) as ps,
            tc.tile_pool(name="pw", bufs=1) as pwp,
            tc.tile_pool(name="psO", bufs=4, space="PSUM") as psO,
        ):
            vcur = pv.tile([128, NBLK * E], f32, tag="va", name="va")
            vnxt = pv.tile([128, NBLK * E], f32, tag="vb", name="vb")
            vbf = pv.tile([128, NBLK * E], bf16, tag="vbf", name="vbf")
            S = [
                [ps.tile([128, PITCH], bf16, tag=f"s{p}_{k}", name=f"s{p}_{k}") for k in range(6)]
                for p in range(2)
            ]
            Wt = pwp.tile([NL, N], bf16, tag="wt", name="wt")
            wt1 = pwp.tile([NL, N], bf16, tag="wt1", name="wt1")

            sv = vstage_out[:].rearrange("a b c -> (a b c)")[0:1]
            sv.ap = V64([[E, 128], [128 * E, NBLK], [1, E]])
            dv = vcur[:].rearrange("p (blk e) -> p blk e", e=E)
            nc.sync.dma_start(out=dv, in_=sv)

            def chain_layer(m, vc, vn):
                st = stage[m % 2]
                Sp = S[m % 2]
                # load this layer's W plain, then (j, b)-interleave on DVE:
                # Wt[l, j*32 + b] = W[128*b + j, l]
                for h2 in range(2):
                    nc.sync.dma_start(
                        out=wt1[:, h2 * ROWS:(h2 + 1) * ROWS], in_=wsos[m][h2]
                    )
                nc.vector.tensor_copy(
                    Wt[:].rearrange("l (j b) -> l j b", b=NBLK),
                    wt1[:].rearrange("l (b j) -> l j b", j=128),
                )
                # rewrite the 13 diagonals of the staged S image (skewed flat APs)
                for li, d in enumerate(OFFS):
                    if d <= 128:
                        segs = []
                        if 128 - d > 0:
                            segs.append((0, 0, 128 - d, d))
                        if d > 0:
                            segs.append((1, 128 - d, d, 0))
                    else:
                        si = {256: 2, 512: 3, 1024: 4, 2048: 5}[d]
                        segs = [(si, 0, 128, 0)]
                    for (si, j0, cnt, p0) in segs:
                        src = Wt[li:li + 1, j0 * NBLK:(j0 + cnt) * NBLK]
                        doff = si * 128 * PITCH + p0 * PITCH + j0 * NBLK
                        dst = st[doff:doff + 1]
                        dst.ap = V64([[PITCH + NBLK, cnt], [1, NBLK]])
                        nc.sync.dma_start(out=dst, in_=src)
                # reload the six S tiles (parity double-buffered)
                for k in range(6):
                    nc.sync.dma_start(
                        out=Sp[k][:],
                        in_=st[k * 128 * PITCH:(k + 1) * 128 * PITCH].rearrange(
                            "(p f) -> p f", f=PITCH
                        ),
                    )
                # bf16 copy of V for the weighted-link matmuls
                nc.vector.tensor_copy(vbf[:], vc[:])
                for blk in range(NBLK):
                    po = psO.tile([128, E], f32, tag="po", name="po")
                    srcs = [(0, blk), (1, (blk + 1) % NBLK)]
                    for i, dl in enumerate([2, 4, 8, 16]):
                        srcs.append((2 + i, (blk + dl) % NBLK))
                    for ii, (si, sb) in enumerate(srcs):
                        nc.tensor.matmul(
                            po[:],
                            lhsT=Sp[si][:, blk::NBLK],
                            rhs=vbf[:, sb * E:(sb + 1) * E],
                            start=(ii == 0),
                            stop=(ii == 5),
                        )
                    # exact f32 residual: V_next = psum + V
                    nc.vector.tensor_add(
                        vn[:, blk * E:(blk + 1) * E],
                        po[:],
                        vc[:, blk * E:(blk + 1) * E],
                    )

            vc, vn = vcur, vnxt
            for m in range(nw):
                chain_layer(m, vc, vn)
                vc, vn = vn, vc

            for t in range(NBLK):
                nc.sync.dma_start(
                    out=out[t * 128:(t + 1) * 128, :], in_=vc[:, t * E:(t + 1) * E]
                )

    _split_multi_waits(nc, mybir)
    return nc


def kernel(**inputs):
    _install_patches()
    from concourse.bass_utils import run_bass_kernel_spmd

    nw = int(os.environ.get("K_NW", NW))
    V = np.ascontiguousarray(np.asarray(inputs["V"], dtype=np.float32))
    inp = np.ascontiguousarray(np.asarray(inputs["input"], dtype=np.float32))
    g_W1 = np.ascontiguousarray(np.asarray(inputs["g_W1"], dtype=np.float32))
    g_b1 = np.asarray(inputs["g_b1"], dtype=np.float32)
    g_W2 = np.ascontiguousarray(np.asarray(inputs["g_W2"], dtype=np.float32))
    g_b2 = np.asarray(inputs["g_b2"], dtype=np.float32)
    f_W1 = np.ascontiguousarray(np.asarray(inputs["f_W1"], dtype=np.float32))[:nw]
    f_b1 = np.asarray(inputs["f_b1"], dtype=np.float32)[:nw]
    f_W2 = np.ascontiguousarray(np.asarray(inputs["f_W2"], dtype=np.float32))[:nw]
    f_b2 = np.asarray(inputs["f_b2"], dtype=np.float32)[:nw]

    import ml_dtypes

    bf = ml_dtypes.bfloat16
    gb1t = np.ascontiguousarray(g_b1.reshape(HT, 128).T)
    fw2t = np.ascontiguousarray(
        f_W2.reshape(nw, HT, 128, NL).transpose(0, 2, 1, 3).reshape(nw, 128, HT * NL)
    ).astype(bf)
    fb1t = np.ascontiguousarray(
        f_b1.reshape(nw, HT, 128).transpose(2, 0, 1).reshape(128, nw * HT)
    )
    fb2c = np.ascontiguousarray(f_b2.T)  # residual added exactly at evacuation

    shared = {
        "gw1": g_W1.astype(bf),
        "gw2": g_W2.astype(bf),
        "gb1t": gb1t,
        "gb2r": np.ascontiguousarray(g_b2[None, :]).astype(bf),
        "onesr": np.ones((1, E), bf),
        "fw1": f_W1.astype(bf),
        "fw2t": fw2t,
        "fb1t": fb1t,
        "fb2c": fb2c,
    }
    in_maps = []
    for c in range(8):
        b, h = c // 2, c % 2
        rows = slice(h * ROWS, (h + 1) * ROWS)
        m = dict(shared)
        m["vt"] = np.ascontiguousarray(V[b, rows].T).astype(bf)
        m["inpt"] = np.ascontiguousarray(inp[b, rows].T).astype(bf)
        in_maps.append(m)

    nc = _build_program(nw)
    trace = bool(int(os.environ.get("K_TRACE", "0")))
    res = run_bass_kernel_spmd(nc, in_maps, list(range(8)), trace=trace)
    kernel.last_result = res

    outp = np.empty((B, N, E), np.float32)
    for b in range(B):
        outp[b] = res.results[2 * b]["out"]
    return outp



# revision 5
# speedup vs baseline: 1.3744x; 1.3744x over previous
"""Trainium2 Bass kernel for nn_AttentionModule_39616778338491 (chord sparse attention).

Structure: V = gMLP(V); 12x { W = fMLP_m(input); V = chord_spmm(W, V) + V }.

Sharding (8 cores): core c = 2b+h -> batch b. f-MLPs are data-parallel over
row halves h; ONE concatenated pair-AllGather shares all 12 layers' W. The
g-MLP and the chord chain are E-split: each core computes the FULL 4096-row
chain on its own 128-column E-half (chord is independent across E), so no V
AllGather and no per-layer halo exchange exist at all.

Program layout: TileContext 1 (12 f-MLPs, bf16 matmuls) -> raw gpsimd block
issuing one AllGather (no wait; Tile's collective path deadlocks here, and
the wait is deferred into phase 2's sync queue so the g-MLP overlaps the
transfer) -> all-engine barrier -> TileContext 2 (g-MLP + chord chain).

Chord spmm as dense PE matmuls: per 128-row output block the 13 power-of-2
offsets touch 6 source blocks {+0,+1,+2,+4,+8,+16}. Slots +0/+1 (links
d<=128, with wraparound spill) are bf16 lhsT images built via skewed flat
DMAs into DRAM (diagonal writes couple partition and byte offsets, which
only DRAM-side APs allow) and reloaded. The four single-diagonal slots
(d=256..2048) never touch DRAM: a [128,32] weight grid is formed by small
SBUF->SBUF DMAs from the interleaved W tile, then one DVE tensor_mul per
slot against a broadcast identity writes the diagonal lhsT tile in SBUF.
The +V residual is an exact-f32 vector-engine add at PSUM evacuation,
grouped 4 output blocks per PSUM bank.
"""

import os
import numpy as np

B, N, E, H = 4, 4096, 256, 1024
NW = 12
NL = 13
OFFS = [0, 1, 2, 4, 8, 16, 32, 64, 128, 256, 512, 1024, 2048]
ROWS = N // 2          # rows per core for f-MLP work
NBLK = N // 128        # 32 blocks of 128 rows
EH = E // 2            # e-half per core for g + chord
CH = 512               # row-chunk for MLP matmuls
NCH = ROWS // CH       # f-MLP chunks (own half)
NCHG = N // CH         # g-MLP chunks (full N)
HT = H // 128          # 8 h-tiles
PITCH = NBLK * 128     # free width of an S/D tile (elems)
GROUPS = [[0, 1], [2, 3], [4, 5], [6, 7]]
DIAG_BOFF = [2, 4, 8, 16]   # links 9..12 (d=256..2048): pure diagonals


def _install_patches():
    """Walrus in this image rejects >1 sem wait on the Tile tail Drain;
    spread the waits across preceding sync-engine nops. Also raise the
    stale SBUF cap (207.87 KB/partition is the real limit here)."""
    import concourse.mybir as mybir
    from concourse.tile import TileContext
    from concourse.vector_clock import ScopedClock
    from concourse import tile_utils

    def _dab(self, tick_clock, wait_clock):
        nops = [self.nc.sync.nop(nofuse=True) for _ in range(27)]
        drain_inst = self.nc.sync.drain()
        wait_clock.add_sem_waits(
            drain_inst.ins, ScopedClock({None: tick_clock.global_clock})
        )
        si = drain_inst.ins.sync_info
        waits = list(si.on_wait) if si else []
        if len(waits) > 1:
            si.on_wait.clear()
            si.on_wait.append(waits[0])
            for w, nop in zip(waits[1:], nops):
                nsi = nop.ins.sync_info
                if nsi is None:
                    nop.ins.sync_info = mybir.SyncInfo(on_update=[], on_wait=[w])
                else:
                    nsi.on_wait.append(w)
        self.nc.all_engine_barrier()
        popped = self.nc._tile_sem_poison_stack.pop()
        assert popped is self._sem_poison
        self.nc.clear_and_free_semaphores(list(self.sems.allocated().values()))
        self.nc.all_engine_barrier()

    TileContext._drain_and_barrier = _dab
    tile_utils.max_sbuf_usage = 206 * 1024


def _split_multi_waits(nc, mybir, limit=1):
    """This walrus build accepts at most one sem wait per instruction;
    hoist extra waits onto same-engine NoOps inserted just before."""
    uid = 0
    for f in nc.m.functions:
        for bb in f.blocks:
            new = []
            for inst in bb.instructions:
                si = inst.sync_info
                waits = list(si.on_wait) if si and si.on_wait else []
                if len(waits) > limit:
                    for w in waits[:-limit]:
                        nop = mybir.InstNoOp(name=f"waitsplit-{uid}", ins=[], outs=[])
                        uid += 1
                        nop.engine = inst.engine
                        nop.sync_info = mybir.SyncInfo(on_update=[], on_wait=[w])
                        new.append(nop)
                    si.on_wait.clear()
                    si.on_wait.append(waits[-1])
                new.append(inst)
            bb.instructions = new


def _build_program(nw):
    import bass_rust
    import concourse.bass as bass
    import concourse.mybir as mybir
    from concourse.tile import TileContext

    f32 = mybir.dt.float32
    bf16 = mybir.dt.bfloat16
    AF = mybir.ActivationFunctionType
    V64 = bass_rust.VecI64Pair

    nc = bass.Bass()
    vtf = nc.declare_dram_parameter("vtf", [E, N], bf16, isOutput=False)
    inpt = nc.declare_dram_parameter("inpt", [E, ROWS], bf16, isOutput=False)
    gw1 = nc.declare_dram_parameter("gw1", [E, H], bf16, isOutput=False)
    gw2h = nc.declare_dram_parameter("gw2h", [H, EH], bf16, isOutput=False)
    gb1t = nc.declare_dram_parameter("gb1t", [128, HT], f32, isOutput=False)
    gb2h = nc.declare_dram_parameter("gb2h", [1, EH], bf16, isOutput=False)
    fw1 = nc.declare_dram_parameter("fw1", [nw, E, H], bf16, isOutput=False)
    fw2t = nc.declare_dram_parameter("fw2t", [nw, 128, HT * NL], bf16, isOutput=False)
    fb1t = nc.declare_dram_parameter("fb1t", [128, nw * HT], f32, isOutput=False)
    fb2c = nc.declare_dram_parameter("fb2c", [NL, nw], f32, isOutput=False)
    onesr = nc.declare_dram_parameter("onesr", [1, 128], bf16, isOutput=False)
    idmr = nc.declare_dram_parameter("idmr", [128, 128], bf16, isOutput=False)
    out = nc.declare_dram_parameter("out", [N, EH], f32, isOutput=True)

    # raw DRAM staging (crosses the TileContext boundary; the phase barrier
    # orders accesses)
    wsis_all = nc.dram_tensor("wsis_all", [nw, NL, ROWS], bf16)
    wsos_all = nc.dram_tensor("wsos_all", [2, nw, NL, ROWS], bf16)
    vstage = nc.dram_tensor("vstage", [N, EH], f32)
    stage = [nc.dram_tensor(f"sst{p}", [2 * 128 * PITCH], bf16) for p in range(2)]

    # ---------------- phase 1: f MLPs ----------------
    with TileContext(nc) as tc:
        with (
            tc.tile_pool(name="pc", bufs=1) as pc,
            tc.tile_pool(name="pin", bufs=1) as pin,
            tc.tile_pool(name="pfh", bufs=1) as pfh,
            tc.tile_pool(name="pfw1", bufs=2) as pfw1,
            tc.tile_pool(name="pfw2", bufs=2) as pfw2,
            tc.tile_pool(name="ptmp", bufs=3) as ptmp,
            tc.tile_pool(name="psA", bufs=3, space="PSUM") as psA,
            tc.tile_pool(name="psW", bufs=2, space="PSUM") as psW,
        ):
            fb1_t = pc.tile([128, nw * HT], f32, tag="fb1", name="fb1")
            fb2_t = pc.tile([NL, nw], f32, tag="fb2", name="fb2")
            inp_t = [pin.tile([128, ROWS], bf16, tag=f"inp{k}", name=f"inp{k}") for k in range(2)]
            zt = pc.tile([128, PITCH], bf16, tag="zt", name="zt")

            for k in range(2):
                nc.sync.dma_start(out=inp_t[k][:], in_=inpt[k * 128:(k + 1) * 128, :])
            nc.sync.dma_start(out=fb1_t[:], in_=fb1t[:])
            nc.sync.dma_start(out=fb2_t[:], in_=fb2c[:])

            # zero the slot-0/1 staging images once (diagonal rewrites never
            # touch the off-diagonal zeros again)
            nc.vector.memset(zt[:], 0.0)
            for par in range(2):
                for k in range(2):
                    nc.sync.dma_start(
                        out=stage[par][k * 128 * PITCH:(k + 1) * 128 * PITCH].rearrange(
                            "(p f) -> p f", f=PITCH
                        ),
                        in_=zt[:],
                    )

            for m in range(nw):
                w1 = [pfw1.tile([128, H], bf16, tag=f"fw1_{k}", name=f"fw1_{k}") for k in range(2)]
                for k in range(2):
                    nc.sync.dma_start(out=w1[k][:], in_=fw1[m, k * 128:(k + 1) * 128, :])
                w2 = pfw2.tile([128, HT * NL], bf16, tag="fw2", name="fw2")
                nc.sync.dma_start(out=w2[:], in_=fw2t[m])
                for ch in range(NCH):
                    fh = [pfh.tile([128, CH], bf16, tag=f"fh{t}", name=f"fh{t}") for t in range(HT)]
                    for ht in range(HT):
                        pa = psA.tile([128, CH], f32, tag="pa", name="pa")
                        for kt in range(2):
                            nc.tensor.matmul(
                                pa[:],
                                lhsT=w1[kt][:, ht * 128:(ht + 1) * 128],
                                rhs=inp_t[kt][:, ch * CH:(ch + 1) * CH],
                                start=(kt == 0),
                                stop=(kt == 1),
                            )
                        nc.scalar.activation(
                            fh[ht][:], pa[:], AF.Gelu,
                            bias=fb1_t[:, m * HT + ht:m * HT + ht + 1],
                        )
                    pw = psW.tile([NL, CH], f32, tag="pw", name="pw")
                    for ht in range(HT):
                        nc.tensor.matmul(
                            pw[:],
                            lhsT=w2[:, ht * NL:(ht + 1) * NL],
                            rhs=fh[ht][:],
                            start=(ht == 0),
                            stop=(ht == HT - 1),
                        )
                    wc = ptmp.tile([NL, CH], bf16, tag="tw", name="tw")
                    nc.vector.tensor_scalar_add(wc[:], pw[:], fb2_t[:, m:m + 1])
                    nc.sync.dma_start(
                        out=wsis_all[m][:, ch * CH:(ch + 1) * CH], in_=wc[:]
                    )

    # ---------------- raw pair-AllGather (issue only; wait in phase 2) ----
    with nc.semaphore("ag_sem") as ag_sem:
        with nc.Block() as blk:
            @blk.gpsimd
            def _(g):
                g.collective_compute(
                    "AllGather", mybir.AluOpType.bypass, replica_groups=GROUPS,
                    ins=[wsis_all[:]], outs=[wsos_all[:]],
                ).then_inc(ag_sem)

        nc.all_engine_barrier()

        # ---------------- phase 2a: g MLP (overlaps the AllGather) --------
        with TileContext(nc) as tc:
            with (
                tc.tile_pool(name="pc2", bufs=1) as pc2,
                tc.tile_pool(name="pvt", bufs=1) as pvt,
                tc.tile_pool(name="pgo", bufs=1) as pgo,
                tc.tile_pool(name="pgh", bufs=1) as pgh,
                tc.tile_pool(name="psA2", bufs=2, space="PSUM") as psA2,
                tc.tile_pool(name="psG", bufs=2, space="PSUM") as psG,
            ):
                gw1_t = [pc2.tile([128, H], bf16, tag=f"gw1_{k}", name=f"gw1_{k}") for k in range(2)]
                gw2_t = pc2.tile([128, HT * EH], bf16, tag="gw2", name="gw2")
                gb1_t = pc2.tile([128, HT], f32, tag="gb1", name="gb1")
                gb2_t = pc2.tile([1, EH], bf16, tag="gb2", name="gb2")
                ones_t = pc2.tile([1, 128], bf16, tag="ones", name="ones")
                vt_t = [pvt.tile([128, N], bf16, tag=f"vt{k}", name=f"vt{k}") for k in range(2)]
                gout = pgo.tile([128, NBLK * EH], f32, tag="go", name="go")

                for k in range(2):
                    nc.sync.dma_start(out=gw1_t[k][:], in_=gw1[k * 128:(k + 1) * 128, :])
                    nc.sync.dma_start(out=vt_t[k][:], in_=vtf[k * 128:(k + 1) * 128, :])
                for t in range(HT):
                    nc.sync.dma_start(
                        out=gw2_t[:, t * EH:(t + 1) * EH],
                        in_=gw2h[t * 128:(t + 1) * 128, :],
                    )
                nc.sync.dma_start(out=gb1_t[:], in_=gb1t[:])
                nc.sync.dma_start(out=gb2_t[:], in_=gb2h[:])
                nc.sync.dma_start(out=ones_t[:], in_=onesr[:])

                # g MLP over all N rows, own E-half -> vstage
                for ch in range(NCHG):
                    fh = [pgh.tile([128, CH], bf16, tag=f"gh{t}", name=f"gh{t}") for t in range(HT)]
                    for ht in range(HT):
                        pa = psA2.tile([128, CH], f32, tag="pa2", name="pa2")
                        for kt in range(2):
                            nc.tensor.matmul(
                                pa[:],
                                lhsT=gw1_t[kt][:, ht * 128:(ht + 1) * 128],
                                rhs=vt_t[kt][:, ch * CH:(ch + 1) * CH],
                                start=(kt == 0),
                                stop=(kt == 1),
                            )
                        nc.scalar.activation(
                            fh[ht][:], pa[:], AF.Gelu, bias=gb1_t[:, ht:ht + 1]
                        )
                    for t in range(4):
                        po = psG.tile([128, EH], f32, tag="pog", name="pog")
                        nc.tensor.matmul(
                            po[:], lhsT=ones_t[0:1, 0:128], rhs=gb2_t[0:1, :],
                            start=True, stop=False,
                        )
                        for ht in range(HT):
                            nc.tensor.matmul(
                                po[:],
                                lhsT=fh[ht][:, t * 128:(t + 1) * 128],
                                rhs=gw2_t[:, ht * EH:(ht + 1) * EH],
                                start=False,
                                stop=(ht == HT - 1),
                            )
                        blk_i = ch * 4 + t
                        nc.scalar.copy(gout[:, blk_i * EH:(blk_i + 1) * EH], po[:])
                nc.sync.dma_start(
                    out=vstage[:].rearrange("(b p) e -> p b e", p=128),
                    in_=gout[:].rearrange("p (b e) -> p b e", e=EH),
                )

        # gate the W-dependent DMA stream on the AllGather (top-level so
        # the Tile scheduler never simulates an externally-signaled wait)
        nc.sync.wait_ge(ag_sem, 1)

        # ---------------- phase 2b: chord chain ----------------
        with TileContext(nc) as tc:
            with (
                tc.tile_pool(name="pc3", bufs=1) as pc3,
                tc.tile_pool(name="pv", bufs=1) as pv,
                tc.tile_pool(name="ps", bufs=2) as ps,
                tc.tile_pool(name="pd", bufs=2) as pd,
                tc.tile_pool(name="pw2", bufs=2) as pw2,
                tc.tile_pool(name="psC", bufs=2, space="PSUM") as psC,
            ):
                idm_t = pc3.tile([128, 128], bf16, tag="idm", name="idm")
                vcur = pv.tile([128, NBLK * EH], f32, tag="va", name="va")
                vnxt = pv.tile([128, NBLK * EH], f32, tag="vb", name="vb")
                vbf = pv.tile([128, NBLK * EH], bf16, tag="vbf", name="vbf")

                nc.sync.dma_start(out=idm_t[:], in_=idmr[:])
                nc.sync.dma_start(
                    out=vcur[:].rearrange("p (b e) -> p b e", e=EH),
                    in_=vstage[:].rearrange("(b p) e -> p b e", p=128),
                )

                def chain_layer(m, vc, vn):
                    st = stage[m % 2]
                    wt1 = pw2.tile([NL, N], bf16, tag="wt1", name="wt1")
                    Wt = pw2.tile([NL, N], bf16, tag="wt", name="wt")
                    WG = pw2.tile([128, 4 * NBLK], bf16, tag="wg", name="wg")
                    # load this layer's W plain, then (j, b)-interleave on DVE:
                    # Wt[l, j*32 + b] = W[128*b + j, l]
                    for h2 in range(2):
                        nc.sync.dma_start(
                            out=wt1[:, h2 * ROWS:(h2 + 1) * ROWS], in_=wsos_all[h2, m]
                        )
                    nc.vector.tensor_copy(
                        Wt[:].rearrange("l (j b) -> l j b", b=NBLK),
                        wt1[:].rearrange("l (b j) -> l j b", j=128),
                    )
                    # weight grid for the 4 pure-diagonal links: WG[p, i*32+b]
                    # = w_{9+i}[b*128+p]  (SBUF->SBUF row spread, no HBM trip)
                    for i in range(4):
                        nc.sync.dma_start(
                            out=WG[:, i * NBLK:(i + 1) * NBLK],
                            in_=Wt[9 + i:10 + i, :].rearrange("o (p b) -> o p b", b=NBLK),
                        )
                    # rewrite the 9 low-link diagonals of the staged S image
                    for li in range(9):
                        d = OFFS[li]
                        segs = []
                        if 128 - d > 0:
                            segs.append((0, 0, 128 - d, d))
                        if d > 0:
                            segs.append((1, 128 - d, d, 0))
                        for (si, j0, cnt, p0) in segs:
                            src = Wt[li:li + 1, j0 * NBLK:(j0 + cnt) * NBLK]
                            doff = si * 128 * PITCH + p0 * PITCH + j0 * NBLK
                            dst = st[doff:doff + 1]
                            dst.ap = V64([[PITCH + NBLK, cnt], [1, NBLK]])
                            nc.sync.dma_start(out=dst, in_=src)
                    # reload the two dense-ish S tiles (parity double-buffered)
                    Sp = []
                    for k in range(2):
                        s = ps.tile([128, PITCH], bf16, tag=f"s{k}", name=f"s{k}")
                        nc.sync.dma_start(
                            out=s[:],
                            in_=st[k * 128 * PITCH:(k + 1) * 128 * PITCH].rearrange(
                                "(p f) -> p f", f=PITCH
                            ),
                        )
                        Sp.append(s)
                    # build the 4 diagonal lhsT tiles on DVE: D[p, (j, b)] =
                    # id[p, j] * WG[p, i*32+b]
                    Dp = []
                    for i in range(4):
                        dt_ = pd.tile([128, PITCH], bf16, tag=f"d{i}", name=f"d{i}")
                        nc.vector.tensor_mul(
                            dt_[:].rearrange("p (j b) -> p j b", b=NBLK),
                            idm_t[:, :, None].broadcast_to((128, 128, NBLK)),
                            WG[:, None, i * NBLK:(i + 1) * NBLK].broadcast_to(
                                (128, 128, NBLK)
                            ),
                        )
                        Dp.append(dt_)
                    # bf16 copy of V for the weighted-link matmuls (scalar
                    # engine; DVE is the busier one in this phase)
                    nc.scalar.copy(vbf[:], vc[:])
                    lhs_all = [Sp[0], Sp[1]] + Dp
                    boffs = [0, 1] + DIAG_BOFF
                    for g4 in range(NBLK // 4):
                        po = psC.tile([128, 4 * EH], f32, tag="poc", name="poc")
                        for i4 in range(4):
                            blk_i = g4 * 4 + i4
                            for ii in range(6):
                                sb = (blk_i + boffs[ii]) % NBLK
                                nc.tensor.matmul(
                                    po[:, i4 * EH:(i4 + 1) * EH],
                                    lhsT=lhs_all[ii][:, blk_i::NBLK],
                                    rhs=vbf[:, sb * EH:(sb + 1) * EH],
                                    start=(ii == 0),
                                    stop=(ii == 5),
                                )
                        # exact f32 residual on 4 grouped blocks
                        nc.vector.tensor_add(
                            vn[:, g4 * 4 * EH:(g4 + 1) * 4 * EH],
                            po[:],
                            vc[:, g4 * 4 * EH:(g4 + 1) * 4 * EH],
                        )

                vc, vn = vcur, vnxt
                for m in range(nw):
                    chain_layer(m, vc, vn)
                    vc, vn = vn, vc

                nc.sync.dma_start(
                    out=out[:].rearrange("(b p) e -> p b e", p=128),
                    in_=vc[:].rearrange("p (b e) -> p b e", e=EH),
                )

    _split_multi_waits(nc, mybir)
    return nc


def kernel(**inputs):
    _install_patches()
    from concourse.bass_utils import run_bass_kernel_spmd

    nw = int(os.environ.get("K_NW", NW))
    V = np.ascontiguousarray(np.asarray(inputs["V"], dtype=np.float32))
    inp = np.ascontiguousarray(np.asarray(inputs["input"], dtype=np.float32))
    g_W1 = np.ascontiguousarray(np.asarray(inputs["g_W1"], dtype=np.float32))
    g_b1 = np.asarray(inputs["g_b1"], dtype=np.float32)
    g_W2 = np.ascontiguousarray(np.asarray(inputs["g_W2"], dtype=np.float32))
    g_b2 = np.asarray(inputs["g_b2"], dtype=np.float32)
    f_W1 = np.ascontiguousarray(np.asarray(inputs["f_W1"], dtype=np.float32))[:nw]
    f_b1 = np.asarray(inputs["f_b1"], dtype=np.float32)[:nw]
    f_W2 = np.ascontiguousarray(np.asarray(inputs["f_W2"], dtype=np.float32))[:nw]
    f_b2 = np.asarray(inputs["f_b2"], dtype=np.float32)[:nw]

    import ml_dtypes

    bf = ml_dtypes.bfloat16
    gb1t = np.ascontiguousarray(g_b1.reshape(HT, 128).T)
    fw2t = np.ascontiguousarray(
        f_W2.reshape(nw, HT, 128, NL).transpose(0, 2, 1, 3).reshape(nw, 128, HT * NL)
    ).astype(bf)
    fb1t = np.ascontiguousarray(
        f_b1.reshape(nw, HT, 128).transpose(2, 0, 1).reshape(128, nw * HT)
    )
    fb2c = np.ascontiguousarray(f_b2.T)  # residual added exactly at evacuation

    shared = {
        "gw1": g_W1.astype(bf),
        "gb1t": gb1t,
        "onesr": np.ones((1, 128), bf),
        "idmr": np.eye(128, dtype=bf),
        "fw1": f_W1.astype(bf),
        "fw2t": fw2t,
        "fb1t": fb1t,
        "fb2c": fb2c,
    }
    in_maps = []
    for c in range(8):
        b, h = c // 2, c % 2
        rows = slice(h * ROWS, (h + 1) * ROWS)
        ecols = slice(h * EH, (h + 1) * EH)
        m = dict(shared)
        m["vtf"] = np.ascontiguousarray(V[b].T).astype(bf)
        m["inpt"] = np.ascontiguousarray(inp[b, rows].T).astype(bf)
        m["gw2h"] = np.ascontiguousarray(g_W2[:, ecols]).astype(bf)
        m["gb2h"] = np.ascontiguousarray(g_b2[None, ecols]).astype(bf)
        in_maps.append(m)

    nc = _build_program(nw)
    trace = bool(int(os.environ.get("K_TRACE", "0")))
    res = run_bass_kernel_spmd(nc, in_maps, list(range(8)), trace=trace)
    kernel.last_result = res

    outp = np.empty((B, N, E), np.float32)
    for b in range(B):
        outp[b, :, :EH] = res.results[2 * b]["out"]
        outp[b, :, EH:] = res.results[2 * b + 1]["out"]
    return outp


# revision 14
# speedup vs baseline: 1.4390x; 1.0470x over previous
"""Trainium2 Bass kernel for nn_AttentionModule_39616778338491 (chord sparse attention).

Structure: V = gMLP(V); 12x { W = fMLP_m(input); V = chord_spmm(W, V) + V }.

Sharding (8 cores): core c = 2b+h -> batch b. f-MLPs are data-parallel over
row halves h; ONE concatenated pair-AllGather shares all 12 layers' W. The
g-MLP and the chord chain are E-split: each core computes the FULL 4096-row
chain on its own 128-column E-half (chord is independent across E), so no V
AllGather and no per-layer halo exchange exist at all.

Program layout: TileContext 1 (12 f-MLPs, bf16 matmuls) -> raw gpsimd block
issuing one AllGather (no wait; Tile's collective path deadlocks here, and
the wait is deferred into phase 2's sync queue so the g-MLP overlaps the
transfer) -> all-engine barrier -> TileContext 2 (g-MLP + chord chain).

Chord spmm as dense PE matmuls: per 128-row output block the 13 power-of-2
offsets touch 6 source blocks {+0,+1,+2,+4,+8,+16}. Slots +0/+1 (links
d<=128, with wraparound spill) are bf16 lhsT images built via skewed flat
DMAs into DRAM (diagonal writes couple partition and byte offsets, which
only DRAM-side APs allow) and reloaded. The four single-diagonal slots
(d=256..2048) never touch DRAM: a [128,32] weight grid is formed by small
SBUF->SBUF DMAs from the interleaved W tile, then one DVE tensor_mul per
slot against a broadcast identity writes the diagonal lhsT tile in SBUF.
The +V residual is an exact-f32 vector-engine add at PSUM evacuation,
grouped 4 output blocks per PSUM bank.
"""

import os
import numpy as np

B, N, E, H = 4, 4096, 256, 1024
NW = 12
NL = 13
OFFS = [0, 1, 2, 4, 8, 16, 32, 64, 128, 256, 512, 1024, 2048]
ROWS = N // 2          # rows per core for f-MLP work
NBLK = N // 128        # 32 blocks of 128 rows
EH = E // 2            # e-half per core for g + chord
CH = 512               # row-chunk for MLP matmuls
NCH = ROWS // CH       # f-MLP chunks (own half)
NCHG = N // CH         # g-MLP chunks (full N)
HT = H // 128          # 8 h-tiles
PITCH = NBLK * 128     # free width of an S/D tile (elems)
GROUPS = [[0, 1], [2, 3], [4, 5], [6, 7]]
DIAG_BOFF = [2, 4, 8, 16]   # links 9..12 (d=256..2048): pure diagonals


def _install_patches():
    """Walrus in this image rejects >1 sem wait on the Tile tail Drain;
    spread the waits across preceding sync-engine nops. Also raise the
    stale SBUF cap (207.87 KB/partition is the real limit here)."""
    import concourse.mybir as mybir
    from concourse.tile import TileContext
    from concourse.vector_clock import ScopedClock
    from concourse import tile_utils

    def _dab(self, tick_clock, wait_clock):
        nops = [self.nc.sync.nop(nofuse=True) for _ in range(27)]
        drain_inst = self.nc.sync.drain()
        wait_clock.add_sem_waits(
            drain_inst.ins, ScopedClock({None: tick_clock.global_clock})
        )
        si = drain_inst.ins.sync_info
        waits = list(si.on_wait) if si else []
        if len(waits) > 1:
            si.on_wait.clear()
            si.on_wait.append(waits[0])
            for w, nop in zip(waits[1:], nops):
                nsi = nop.ins.sync_info
                if nsi is None:
                    nop.ins.sync_info = mybir.SyncInfo(on_update=[], on_wait=[w])
                else:
                    nsi.on_wait.append(w)
        self.nc.all_engine_barrier()
        popped = self.nc._tile_sem_poison_stack.pop()
        assert popped is self._sem_poison
        self.nc.clear_and_free_semaphores(list(self.sems.allocated().values()))
        self.nc.all_engine_barrier()

    TileContext._drain_and_barrier = _dab
    tile_utils.max_sbuf_usage = 206 * 1024


def _split_multi_waits(nc, mybir, limit=1):
    """This walrus build accepts at most one sem wait per instruction;
    hoist extra waits onto same-engine NoOps inserted just before."""
    uid = 0
    for f in nc.m.functions:
        for bb in f.blocks:
            new = []
            for inst in bb.instructions:
                si = inst.sync_info
                waits = list(si.on_wait) if si and si.on_wait else []
                if len(waits) > limit:
                    for w in waits[:-limit]:
                        nop = mybir.InstNoOp(name=f"waitsplit-{uid}", ins=[], outs=[])
                        uid += 1
                        nop.engine = inst.engine
                        nop.sync_info = mybir.SyncInfo(on_update=[], on_wait=[w])
                        new.append(nop)
                    si.on_wait.clear()
                    si.on_wait.append(waits[-1])
                new.append(inst)
            bb.instructions = new


def _build_program(nw):
    import bass_rust
    import concourse.bass as bass
    import concourse.mybir as mybir
    from concourse.tile import TileContext

    f32 = mybir.dt.float32
    bf16 = mybir.dt.bfloat16
    AF = mybir.ActivationFunctionType
    V64 = bass_rust.VecI64Pair

    nc = bass.Bass()
    vtf = nc.declare_dram_parameter("vtf", [E, N], bf16, isOutput=False)
    inpt = nc.declare_dram_parameter("inpt", [E, ROWS], bf16, isOutput=False)
    gw1 = nc.declare_dram_parameter("gw1", [E, H], bf16, isOutput=False)
    gw2h = nc.declare_dram_parameter("gw2h", [H, EH], bf16, isOutput=False)
    gb1t = nc.declare_dram_parameter("gb1t", [128, HT], f32, isOutput=False)
    gb2h = nc.declare_dram_parameter("gb2h", [1, EH], bf16, isOutput=False)
    fw1 = nc.declare_dram_parameter("fw1", [nw, E, H], bf16, isOutput=False)
    fw2t = nc.declare_dram_parameter("fw2t", [nw, 128, HT * NL], bf16, isOutput=False)
    fb1t = nc.declare_dram_parameter("fb1t", [128, nw * HT], f32, isOutput=False)
    fb2c = nc.declare_dram_parameter("fb2c", [NL, nw], f32, isOutput=False)
    onesr = nc.declare_dram_parameter("onesr", [1, 128], bf16, isOutput=False)
    idmr = nc.declare_dram_parameter("idmr", [128, 128], bf16, isOutput=False)
    out = nc.declare_dram_parameter("out", [N, EH], f32, isOutput=True)

    # raw DRAM staging (crosses the TileContext boundary; the phase barrier
    # orders accesses)
    split = min(2, nw)
    wsis_all = nc.dram_tensor("wsis_all", [nw, NL, ROWS], bf16)
    wsos_a = nc.dram_tensor("wsos_a", [2, split, NL, ROWS], bf16)
    wsos_b = (
        nc.dram_tensor("wsos_b", [2, nw - split, NL, ROWS], bf16)
        if nw > split else None
    )
    vstage = nc.dram_tensor("vstage", [N, EH], f32)
    stage = [nc.dram_tensor(f"sst{p}", [2 * 128 * PITCH], bf16) for p in range(2)]

    # ---------------- phase 1: f MLPs ----------------
    with TileContext(nc) as tc:
        with (
            tc.tile_pool(name="pc", bufs=1) as pc,
            tc.tile_pool(name="pin", bufs=1) as pin,
            tc.tile_pool(name="pfh", bufs=1) as pfh,
            tc.tile_pool(name="pfw1", bufs=2) as pfw1,
            tc.tile_pool(name="pfw2", bufs=2) as pfw2,
            tc.tile_pool(name="ptmp", bufs=3) as ptmp,
            tc.tile_pool(name="psA", bufs=3, space="PSUM") as psA,
            tc.tile_pool(name="psW", bufs=2, space="PSUM") as psW,
        ):
            fb1_t = pc.tile([128, nw * HT], f32, tag="fb1", name="fb1")
            fb2_t = pc.tile([NL, nw], f32, tag="fb2", name="fb2")
            inp_t = [pin.tile([128, ROWS], bf16, tag=f"inp{k}", name=f"inp{k}") for k in range(2)]
            zt = pc.tile([128, PITCH], bf16, tag="zt", name="zt")

            for k in range(2):
                nc.sync.dma_start(out=inp_t[k][:], in_=inpt[k * 128:(k + 1) * 128, :])
            nc.sync.dma_start(out=fb1_t[:], in_=fb1t[:])
            nc.sync.dma_start(out=fb2_t[:], in_=fb2c[:])

            # zero the slot-0/1 staging images once (diagonal rewrites never
            # touch the off-diagonal zeros again)
            nc.vector.memset(zt[:], 0.0)
            for par in range(2):
                for k in range(2):
                    nc.sync.dma_start(
                        out=stage[par][k * 128 * PITCH:(k + 1) * 128 * PITCH].rearrange(
                            "(p f) -> p f", f=PITCH
                        ),
                        in_=zt[:],
                    )

            for m in range(nw):
                w1 = [pfw1.tile([128, H], bf16, tag=f"fw1_{k}", name=f"fw1_{k}") for k in range(2)]
                for k in range(2):
                    nc.sync.dma_start(out=w1[k][:], in_=fw1[m, k * 128:(k + 1) * 128, :])
                w2 = pfw2.tile([128, HT * NL], bf16, tag="fw2", name="fw2")
                nc.sync.dma_start(out=w2[:], in_=fw2t[m])
                for ch in range(NCH):
                    fh = [pfh.tile([128, CH], bf16, tag=f"fh{t}", name=f"fh{t}") for t in range(HT)]
                    for ht in range(HT):
                        pa = psA.tile([128, CH], f32, tag="pa", name="pa")
                        for kt in range(2):
                            nc.tensor.matmul(
                                pa[:],
                                lhsT=w1[kt][:, ht * 128:(ht + 1) * 128],
                                rhs=inp_t[kt][:, ch * CH:(ch + 1) * CH],
                                start=(kt == 0),
                                stop=(kt == 1),
                            )
                        nc.scalar.activation(
                            fh[ht][:], pa[:], AF.Gelu,
                            bias=fb1_t[:, m * HT + ht:m * HT + ht + 1],
                        )
                    pw = psW.tile([NL, CH], f32, tag="pw", name="pw")
                    for ht in range(HT):
                        nc.tensor.matmul(
                            pw[:],
                            lhsT=w2[:, ht * NL:(ht + 1) * NL],
                            rhs=fh[ht][:],
                            start=(ht == 0),
                            stop=(ht == HT - 1),
                        )
                    wc = ptmp.tile([NL, CH], bf16, tag="tw", name="tw")
                    nc.vector.tensor_scalar_add(wc[:], pw[:], fb2_t[:, m:m + 1])
                    nc.sync.dma_start(
                        out=wsis_all[m][:, ch * CH:(ch + 1) * CH], in_=wc[:]
                    )

    # ---------------- raw pair-AllGathers (issue only; wait in phase 2) ---
    # Split so the first chord layers' W lands early: the collective engine
    # moves ~8 GB/s, so one big AG would finish well after the g-MLP.
    with nc.semaphore("ag_sem") as ag_sem:
        with nc.Block() as blk:
            @blk.gpsimd
            def _(g):
                g.collective_compute(
                    "AllGather", mybir.AluOpType.bypass, replica_groups=GROUPS,
                    ins=[wsis_all[0:split]], outs=[wsos_a[:]],
                ).then_inc(ag_sem)
                if wsos_b is not None:
                    g.collective_compute(
                        "AllGather", mybir.AluOpType.bypass, replica_groups=GROUPS,
                        ins=[wsis_all[split:nw]], outs=[wsos_b[:]],
                    ).then_inc(ag_sem)

        nc.all_engine_barrier()

        # ---------------- phase 2a: g MLP (overlaps the AllGather) --------
        with TileContext(nc) as tc:
            with (
                tc.tile_pool(name="pc2", bufs=1) as pc2,
                tc.tile_pool(name="pvt", bufs=1) as pvt,
                tc.tile_pool(name="pgo", bufs=1) as pgo,
                tc.tile_pool(name="pgh", bufs=1) as pgh,
                tc.tile_pool(name="psA2", bufs=2, space="PSUM") as psA2,
                tc.tile_pool(name="psG", bufs=2, space="PSUM") as psG,
            ):
                gw1_t = [pc2.tile([128, H], bf16, tag=f"gw1_{k}", name=f"gw1_{k}") for k in range(2)]
                gw2_t = pc2.tile([128, HT * EH], bf16, tag="gw2", name="gw2")
                gb1_t = pc2.tile([128, HT], f32, tag="gb1", name="gb1")
                gb2_t = pc2.tile([1, EH], bf16, tag="gb2", name="gb2")
                ones_t = pc2.tile([1, 128], bf16, tag="ones", name="ones")
                vt_t = [pvt.tile([128, N], bf16, tag=f"vt{k}", name=f"vt{k}") for k in range(2)]
                gout = pgo.tile([128, NBLK * EH], f32, tag="go", name="go")

                for k in range(2):
                    nc.sync.dma_start(out=gw1_t[k][:], in_=gw1[k * 128:(k + 1) * 128, :])
                    nc.sync.dma_start(out=vt_t[k][:], in_=vtf[k * 128:(k + 1) * 128, :])
                for t in range(HT):
                    nc.sync.dma_start(
                        out=gw2_t[:, t * EH:(t + 1) * EH],
                        in_=gw2h[t * 128:(t + 1) * 128, :],
                    )
                nc.sync.dma_start(out=gb1_t[:], in_=gb1t[:])
                nc.sync.dma_start(out=gb2_t[:], in_=gb2h[:])
                nc.sync.dma_start(out=ones_t[:], in_=onesr[:])

                # g MLP over all N rows, own E-half -> vstage
                for ch in range(NCHG):
                    fh = [pgh.tile([128, CH], bf16, tag=f"gh{t}", name=f"gh{t}") for t in range(HT)]
                    for ht in range(HT):
                        pa = psA2.tile([128, CH], f32, tag="pa2", name="pa2")
                        for kt in range(2):
                            nc.tensor.matmul(
                                pa[:],
                                lhsT=gw1_t[kt][:, ht * 128:(ht + 1) * 128],
                                rhs=vt_t[kt][:, ch * CH:(ch + 1) * CH],
                                start=(kt == 0),
                                stop=(kt == 1),
                            )
                        nc.scalar.activation(
                            fh[ht][:], pa[:], AF.Gelu, bias=gb1_t[:, ht:ht + 1]
                        )
                    for t in range(4):
                        po = psG.tile([128, EH], f32, tag="pog", name="pog")
                        nc.tensor.matmul(
                            po[:], lhsT=ones_t[0:1, 0:128], rhs=gb2_t[0:1, :],
                            start=True, stop=False,
                        )
                        for ht in range(HT):
                            nc.tensor.matmul(
                                po[:],
                                lhsT=fh[ht][:, t * 128:(t + 1) * 128],
                                rhs=gw2_t[:, ht * EH:(ht + 1) * EH],
                                start=False,
                                stop=(ht == HT - 1),
                            )
                        blk_i = ch * 4 + t
                        nc.scalar.copy(gout[:, blk_i * EH:(blk_i + 1) * EH], po[:])
                nc.sync.dma_start(
                    out=vstage[:].rearrange("(b p) e -> p b e", p=128),
                    in_=gout[:].rearrange("p (b e) -> p b e", e=EH),
                )

        # gate the W-dependent DMA stream on the AllGather (top-level so
        # the Tile scheduler never simulates an externally-signaled wait)
        nc.sync.wait_ge(ag_sem, 1)

        # ---------------- phase 2b: chord chain ----------------
        with TileContext(nc) as tc:
            with (
                tc.tile_pool(name="pc3", bufs=1) as pc3,
                tc.tile_pool(name="pv", bufs=1) as pv,
                tc.tile_pool(name="ps", bufs=2) as ps,
                tc.tile_pool(name="pd", bufs=2) as pd,
                tc.tile_pool(name="pw2", bufs=2) as pw2,
                tc.tile_pool(name="psC", bufs=1, space="PSUM") as psC,
            ):
                idm_t = pc3.tile([128, 128], bf16, tag="idm", name="idm")
                vcur = pv.tile([128, NBLK * EH], f32, tag="va", name="va")
                vnxt = pv.tile([128, NBLK * EH], f32, tag="vb", name="vb")
                vbf = pv.tile([128, NBLK * EH], bf16, tag="vbf", name="vbf")

                nc.sync.dma_start(out=idm_t[:], in_=idmr[:])
                nc.sync.dma_start(
                    out=vcur[:].rearrange("p (b e) -> p b e", e=EH),
                    in_=vstage[:].rearrange("(b p) e -> p b e", p=128),
                )

                agb_waiter = []

                def chain_layer(m, vc, vn):
                    st = stage[m % 2]
                    wt1 = pw2.tile([NL, N], bf16, tag="wt1", name="wt1")
                    Wt = pw2.tile([NL, N], bf16, tag="wt", name="wt")
                    WG = pw2.tile([128, 4 * NBLK], bf16, tag="wg", name="wg")
                    # load this layer's W plain, then (j, b)-interleave on DVE:
                    # Wt[l, j*32 + b] = W[128*b + j, l]
                    for h2 in range(2):
                        wsrc = wsos_a[h2, m] if m < split else wsos_b[h2, m - split]
                        inst = nc.sync.dma_start(
                            out=wt1[:, h2 * ROWS:(h2 + 1) * ROWS], in_=wsrc
                        )
                        if m == split and h2 == 0:
                            # patched post-scheduling: wait for the second AG
                            agb_waiter.append(inst)
                    nc.vector.tensor_copy(
                        Wt[:].rearrange("l (j b) -> l j b", b=NBLK),
                        wt1[:].rearrange("l (b j) -> l j b", j=128),
                    )
                    # weight grid for the 4 pure-diagonal links: WG[p, i*32+b]
                    # = w_{9+i}[b*128+p]  (SBUF->SBUF row spread, no HBM trip)
                    for i in range(4):
                        nc.sync.dma_start(
                            out=WG[:, i * NBLK:(i + 1) * NBLK],
                            in_=Wt[9 + i:10 + i, :].rearrange("o (p b) -> o p b", b=NBLK),
                        )
                    # rewrite the 9 low-link diagonals of the staged S image
                    # (link 0 carries the +1 residual, folded into fb2c on host)
                    for li in range(9):
                        d = OFFS[li]
                        segs = []
                        if 128 - d > 0:
                            segs.append((0, 0, 128 - d, d))
                        if d > 0:
                            segs.append((1, 128 - d, d, 0))
                        for (si, j0, cnt, p0) in segs:
                            src = Wt[li:li + 1, j0 * NBLK:(j0 + cnt) * NBLK]
                            doff = si * 128 * PITCH + p0 * PITCH + j0 * NBLK
                            dst = st[doff:doff + 1]
                            dst.ap = V64([[PITCH + NBLK, cnt], [1, NBLK]])
                            nc.sync.dma_start(out=dst, in_=src)
                    # reload the two dense-ish S tiles (parity double-buffered)
                    Sp = []
                    for k in range(2):
                        s = ps.tile([128, PITCH], bf16, tag=f"s{k}", name=f"s{k}")
                        nc.sync.dma_start(
                            out=s[:],
                            in_=st[k * 128 * PITCH:(k + 1) * 128 * PITCH].rearrange(
                                "(p f) -> p f", f=PITCH
                            ),
                        )
                        Sp.append(s)
                    # build the 4 diagonal lhsT tiles on DVE with contiguous
                    # inner dim: D[p, (b, j)] = id[p, j] * WG[p, i*32+b]
                    Dp = []
                    for i in range(4):
                        dt_ = pd.tile([128, PITCH], bf16, tag=f"d{i}", name=f"d{i}")
                        nc.vector.tensor_mul(
                            dt_[:].rearrange("p (b j) -> p b j", j=128),
                            idm_t[:, None, :].broadcast_to((128, NBLK, 128)),
                            WG[:, i * NBLK:(i + 1) * NBLK][:, :, None].broadcast_to(
                                (128, NBLK, 128)
                            ),
                        )
                        Dp.append(dt_)
                    # bf16 copy of V for the weighted-link matmuls (scalar
                    # engine; per-region copies raced the slot-0 matmuls)
                    nc.scalar.copy(vbf[:], vc[:])
                    boffs = [0, 1] + DIAG_BOFF
                    po8 = [
                        psC.tile([128, 4 * EH], f32, tag=f"poc{g4}", name=f"poc{g4}")
                        for g4 in range(NBLK // 4)
                    ]
                    # block-major: a region's 6-matmul accumulation group runs
                    # uninterrupted — first_mm clears the whole bank's
                    # has_written bits, so interleaving groups within a bank
                    # silently turns accumulates into overwrites
                    for g4 in range(NBLK // 4):
                        for i4 in range(4):
                            blk_i = g4 * 4 + i4
                            for ii in range(6):
                                sb = (blk_i + boffs[ii]) % NBLK
                                if ii < 2:
                                    lhsT = Sp[ii][:, blk_i::NBLK]
                                else:
                                    lhsT = Dp[ii - 2][:, blk_i * 128:(blk_i + 1) * 128]
                                nc.tensor.matmul(
                                    po8[g4][:, i4 * EH:(i4 + 1) * EH],
                                    lhsT=lhsT,
                                    rhs=vbf[:, sb * EH:(sb + 1) * EH],
                                    start=(ii == 0),
                                    stop=(ii == 5),
                                )
                    # evacuate; residual already folded into link 0's weights
                    for g4 in range(NBLK // 4):
                        nc.scalar.copy(
                            vn[:, g4 * 4 * EH:(g4 + 1) * 4 * EH], po8[g4][:]
                        )

                vc, vn = vcur, vnxt
                for m in range(nw):
                    chain_layer(m, vc, vn)
                    vc, vn = vn, vc

                nc.sync.dma_start(
                    out=out[:].rearrange("(b p) e -> p b e", p=128),
                    in_=vc[:].rearrange("p (b e) -> p b e", e=EH),
                )

        # now that Tile scheduling is done, gate layer-`split`'s W load on
        # the second AllGather (an in-context wait on an externally-signaled
        # sem would deadlock the Tile scheduling simulator)
        for inst in agb_waiter:
            inst.wait_op(ag_sem, 2, "sem-ge", check=False)

    _split_multi_waits(nc, mybir)
    return nc


def kernel(**inputs):
    _install_patches()
    from concourse.bass_utils import run_bass_kernel_spmd

    nw = int(os.environ.get("K_NW", NW))
    V = np.ascontiguousarray(np.asarray(inputs["V"], dtype=np.float32))
    inp = np.ascontiguousarray(np.asarray(inputs["input"], dtype=np.float32))
    g_W1 = np.ascontiguousarray(np.asarray(inputs["g_W1"], dtype=np.float32))
    g_b1 = np.asarray(inputs["g_b1"], dtype=np.float32)
    g_W2 = np.ascontiguousarray(np.asarray(inputs["g_W2"], dtype=np.float32))
    g_b2 = np.asarray(inputs["g_b2"], dtype=np.float32)
    f_W1 = np.ascontiguousarray(np.asarray(inputs["f_W1"], dtype=np.float32))[:nw]
    f_b1 = np.asarray(inputs["f_b1"], dtype=np.float32)[:nw]
    f_W2 = np.ascontiguousarray(np.asarray(inputs["f_W2"], dtype=np.float32))[:nw]
    f_b2 = np.asarray(inputs["f_b2"], dtype=np.float32)[:nw]

    import ml_dtypes

    bf = ml_dtypes.bfloat16
    gb1t = np.ascontiguousarray(g_b1.reshape(HT, 128).T)
    fw2t = np.ascontiguousarray(
        f_W2.reshape(nw, HT, 128, NL).transpose(0, 2, 1, 3).reshape(nw, 128, HT * NL)
    ).astype(bf)
    fb1t = np.ascontiguousarray(
        f_b1.reshape(nw, HT, 128).transpose(2, 0, 1).reshape(128, nw * HT)
    )
    fb2c = np.ascontiguousarray(f_b2.T).copy()
    fb2c[0, :] += 1.0  # fold the +V residual into the self-link weight

    shared = {
        "gw1": g_W1.astype(bf),
        "gb1t": gb1t,
        "onesr": np.ones((1, 128), bf),
        "idmr": np.eye(128, dtype=bf),
        "fw1": f_W1.astype(bf),
        "fw2t": fw2t,
        "fb1t": fb1t,
        "fb2c": fb2c,
    }
    in_maps = []
    for c in range(8):
        b, h = c // 2, c % 2
        rows = slice(h * ROWS, (h + 1) * ROWS)
        ecols = slice(h * EH, (h + 1) * EH)
        m = dict(shared)
        m["vtf"] = np.ascontiguousarray(V[b].T).astype(bf)
        m["inpt"] = np.ascontiguousarray(inp[b, rows].T).astype(bf)
        m["gw2h"] = np.ascontiguousarray(g_W2[:, ecols]).astype(bf)
        m["gb2h"] = np.ascontiguousarray(g_b2[None, ecols]).astype(bf)
        in_maps.append(m)

    nc = _build_program(nw)
    trace = bool(int(os.environ.get("K_TRACE", "0")))
    res = run_bass_kernel_spmd(nc, in_maps, list(range(8)), trace=trace)
    kernel.last_result = res

    outp = np.empty((B, N, E), np.float32)
    for b in range(B):
        outp[b, :, :EH] = res.results[2 * b]["out"]
        outp[b, :, EH:] = res.results[2 * b + 1]["out"]
    return outp
